# revision 3
# baseline (speedup 1.0000x reference)
"""BitNet decoder MLP on 8 Trainium2 NeuronCores (Bass/Tile).

Strategy: data-parallel over batch (512 rows/core). Weights are ternary-
quantized cooperatively: each core streams its 1/8 of the fp32 weight
chunks through a |W| pass (DVE for L0/L1, gpsimd for L2/L3, reads split
across two DMA queues so the AllReduce triggers at the barrier floor),
one [128,4] AllReduce covers all four layers' global |W| sums, the
cross-partition total comes from a ones-matmul on the idle tensor
engine. L0/L1 are then re-read + quantized to fp8 ternary on scalar/DVE
and AllGathered immediately; L2/L3 quant runs entirely on the gpsimd
engine (ALU + its own DMA queue) and is EMITTED BETWEEN main-pass layer
emissions so its work never head-of-line-blocks the scalar/vector FIFOs
that the main pass needs (this blocking cost the previous revision
~200us: main-pass evictions sat behind DMA-gated quant ACTs).

Matmuls are bf16 int8-valued activations (stationary) x fp8 ternary
weights (moving), fp32 PSUM accumulation - arithmetic exact. fp8
activations (DoubleRow 2x) were measured numerically infeasible: the
reference pipeline amplifies any per-layer perturbation via act-quant
boundary flips (even an exact f32 reimplementation sits at ~1.1e-2 of
the 2e-2 budget; fp8 acts land at 2-5e-2).

Main pass is pipelined at half-batch granularity: each layer runs two
sweeps of 2 batch tiles; while sweep B's matmuls run, sweep A's tail
executes on scalar/vector engines. The tail is fused: one scalar ACT
computes silu((y-mu)*istd) directly (Silu af with per-partition
bias/scale), then absmax -> scale -> one ACT + one DVE op requantize to
bf16, and a DMA-transpose produces the next layer's stationary tiles.
Per-layer AllGathers fire as soon as each half is quantized; the CC
stream order is L0, L1h0, L1h1, L2h0, L2h1, L3 so each layer's image
lands before its matmuls need it.

Weight images are unit-major ("unit" = [128, panel_ic*512] block with
contiguous per-partition rows) so big DMAs move >=8KB-contiguous lines.
"""

import numpy as np

import concourse.bass as bass
import concourse.mybir as mybir
import concourse.tile as tile
from concourse import bacc, bass_isa
from concourse.bass_utils import run_bass_kernel_spmd

F32 = mybir.dt.float32
BF16 = mybir.dt.bfloat16
FP8 = mybir.dt.float8e4
AF = mybir.ActivationFunctionType
OP = mybir.AluOpType

N_CORES = 8
P = 128
OBW = 512            # output block width (one PSUM bank of fp32)
CH_ELS = P * OBW     # elements per weight chunk
MAGIC = 12582912.0   # 1.5 * 2**23: fp32 round-to-nearest-even trick
EPS = 1e-5
RUN = 4              # chunks per weight-pass slice ([128, 2048] f32)
SILU_AF = True       # fused Silu activation table (False: tanh compose)

FULL_CFG = dict(B=4096, D0=1024, H=4096, OBINS=1000)


def _plan(cfg):
    """Static per-layer plan."""
    B, D0, H, OBINS = cfg["B"], cfg["D0"], cfg["H"], cfg["OBINS"]
    o3_real = 2 * OBINS
    o3_pad = ((o3_real + OBW - 1) // OBW) * OBW
    dims = [
        dict(din=D0, dout=H, dreal=H),
        dict(din=H, dout=H, dreal=H),
        dict(din=H, dout=H, dreal=H),
        dict(din=H, dout=o3_pad, dreal=o3_real),
    ]
    numels = [H * D0, H * H, H * H, o3_real * H]  # real numels for mean|W|
    layers = []
    ch_base = 0
    for li, d in enumerate(dims):
        n_ic = d["din"] // P
        n_ob = d["dout"] // OBW
        n_ch = n_ob * n_ic
        assert n_ch % N_CORES == 0, (li, n_ch)
        panel_ic = min(16, n_ic, max(1, n_ch // N_CORES))
        assert n_ic % panel_ic == 0 and (n_ch // N_CORES) % panel_ic == 0
        n_panels = n_ic // panel_ic
        layers.append(dict(
            li=li, din=d["din"], dout=d["dout"], dreal=d["dreal"],
            n_ic=n_ic, n_ob=n_ob, n_ch=n_ch, per_rank=n_ch // N_CORES,
            panel_ic=panel_ic, n_panels=n_panels,
            numel=numels[li], ch_base=ch_base,
            n_halves=2 if (n_ch // N_CORES) % 2 == 0 and (n_ch // N_CORES) // 2 % panel_ic == 0 else 1,
            ob_w=[min(OBW, d["dreal"] - ob * OBW) for ob in range(n_ob)],
        ))
        ch_base += n_ch
    total_ch = ch_base
    per_rank = total_ch // N_CORES
    b_core = B // N_CORES
    assert b_core % P == 0
    return layers, total_ch, per_rank, b_core // P


def _rsqrt_newton(nc, pool, v, n_iter=3):
    """istd = 1/sqrt(v) for v [128,1] fp32 (v > 0), pure-DVE Newton iteration."""
    seed = pool.tile([P, 1], F32, tag="rs_seed", name="rs_seed")
    seed_i32 = seed[:].bitcast(mybir.dt.int32)
    v_i32 = v.bitcast(mybir.dt.int32)
    nc.vector.tensor_scalar(seed_i32[:], v_i32[:], -0.5,
                            float(0x5F370000), OP.mult, OP.add)
    y = seed
    t1 = pool.tile([P, 1], F32, tag="rs_t1", name="rs_t1")
    t2 = pool.tile([P, 1], F32, tag="rs_t2", name="rs_t2")
    for _ in range(n_iter):
        nc.vector.tensor_tensor(t1[:], y[:], y[:], OP.mult)
        nc.vector.tensor_tensor(t2[:], t1[:], v, OP.mult)
        nc.vector.tensor_scalar(t1[:], t2[:], -0.5, 1.5, OP.mult, OP.add)
        nc.vector.tensor_tensor(y[:], y[:], t1[:], OP.mult)
    return y


def build(cfg):
    layers, total_ch, per_rank, T = _plan(cfg)
    nc = bacc.Bacc("TRN2", target_bir_lowering=False, debug=False,
                   num_devices=N_CORES)

    D0, OBINS = cfg["D0"], cfg["OBINS"]
    b_core = T * P
    SW = min(2, T)  # batch tiles per PE sweep

    xs = nc.dram_tensor("xs", [b_core, D0], F32, kind="ExternalInput")
    # rank's weight chunks, unit-major flat fp32 (see prepare_inputs)
    wsh = nc.dram_tensor("wsh", [per_rank * CH_ELS], F32, kind="ExternalInput")
    mz_out = nc.dram_tensor("mz", [b_core, OBINS], F32, kind="ExternalOutput")
    ii_out = nc.dram_tensor("ii", [b_core, OBINS], F32, kind="ExternalOutput")

    with tile.TileContext(nc) as tc:
        with (
            tc.tile_pool(name="ybig", bufs=4) as ypool,        # 16KB/part f32
            tc.tile_pool(name="wr", bufs=3) as wrpool,         # [128,2048] f32 runs
            tc.tile_pool(name="xqT", bufs=4) as xqTpool,       # [128,32,128] bf16
            tc.tile_pool(name="xqT0", bufs=4) as xqT0pool,     # [128,n_ic0,128] bf16
            tc.tile_pool(name="wp", bufs=3) as wpool,          # [128,16,512] fp8
            tc.tile_pool(name="xqn", bufs=2) as xqnpool,       # 8KB/partition bf16
            tc.tile_pool(name="u", bufs=2) as upool,           # [128,512] f32
            tc.tile_pool(name="sg", bufs=2) as sgpool,         # [128,512] f32
            tc.tile_pool(name="q8", bufs=2) as q8pool,         # [128,2048] fp8
            tc.tile_pool(name="outr", bufs=2) as outpool,      # [128,512] f32
            tc.tile_pool(name="small", bufs=1) as small,
            tc.tile_pool(name="psum", bufs=7, space="PSUM") as psum,
            tc.tile_pool(name="psc", bufs=1, space="PSUM") as pscale,
            tc.tile_pool(name="dram", bufs=1, space="DRAM") as dram,
        ):
            # ---------------- DRAM scratch (flat, unit-major) ----------------
            stage = []
            image = []
            for L in layers:
                nh = L["n_halves"]
                hs = L["per_rank"] // nh * CH_ELS
                hi = L["n_ch"] // nh * CH_ELS
                stage.append([dram.tile([hs], FP8, tag=f"stage{L['li']}_{h}",
                                        name=f"stage{L['li']}_{h}")
                              for h in range(nh)])
                image.append([dram.tile([hi], FP8, tag=f"image{L['li']}_{h}",
                                        name=f"image{L['li']}_{h}",
                                        addr_space="Shared")
                              for h in range(nh)])
            ar_in = dram.tile([P, 4], F32, tag="ar_in", name="ar_in")
            ar_out = dram.tile([P, 4], F32, tag="ar_out", name="ar_out",
                               addr_space="Shared")

            layer_jofs = {}
            jofs = 0
            for L in layers:
                layer_jofs[L["li"]] = jofs
                jofs += L["per_rank"]

            # ---------------- Stage A: input activation quant ----------------
            # x loads ride the scalar DMA queue so the sync queue belongs to
            # the weight-abs reads from t=0.
            n_ic0 = layers[0]["n_ic"]
            xqT_cur = []
            am0s = []
            for t in range(T):
                xt = ypool.tile([P, D0], F32, tag="y", name=f"xt{t}")
                nc.scalar.dma_start(xt[:], xs[t * P:(t + 1) * P, :])
                am = small.tile([P, 1], F32, tag=f"am0_{t}", name=f"am0_{t}")
                nc.vector.tensor_reduce(am[:], xt[:], mybir.AxisListType.X,
                                        OP.max, apply_absolute_value=True)
                nc.vector.tensor_scalar(am[:], am[:], float(EPS), None, OP.max)
                sc = small.tile([P, 1], F32, tag=f"s0_{t}", name=f"s0_{t}")
                nc.vector.tensor_scalar(sc[:], am[:], 1.0 / 127.0, None, OP.mult)
                nc.vector.reciprocal(sc[:], sc[:])
                xq0 = xqnpool.tile([P, D0], BF16, tag="xqn", name=f"xq0_{t}")
                for ch in range(D0 // OBW):
                    uu = upool.tile([P, OBW], F32, tag="u", name=f"u0_{t}_{ch}")
                    nc.scalar.activation(uu[:], xt[:, ch * OBW:(ch + 1) * OBW],
                                         AF.Copy, bias=MAGIC, scale=sc[:])
                    nc.vector.tensor_scalar(xq0[:, ch * OBW:(ch + 1) * OBW],
                                            uu[:], MAGIC, None, OP.subtract)
                xqT0 = xqT0pool.tile([P, n_ic0, P], BF16, tag="xqT0",
                                     name=f"xqT0_{t}")
                nc.scalar.dma_start_transpose(xqT0[:], xq0[:])
                xqT_cur.append(xqT0)
                am0s.append(am)

            # ---------- Stage B1: |W| pass for all layers ----------
            # Reads alternate between the sync and scalar DMA queues (a single
            # queue tops out ~220GB/s; two get the 23MB shard read done by
            # ~worst(core-start skew)). L0/L1 reduce on DVE, L2/L3 on gpsimd.
            ones = small.tile([P, P], F32, tag="ones", name="ones")
            nc.vector.memset(ones[:], 1.0)

            pm4 = small.tile([P, 4], F32, tag="pm4", name="pm4")
            dma_engs = [nc.sync, nc.scalar]
            n_dma = 0
            for li in range(4):
                L = layers[li]
                pr, ljofs = L["per_rank"], layer_jofs[li]
                eng = nc.vector  # gpsimd tensor_reduce lacks free-axis mode
                n_run = (pr + RUN - 1) // RUN
                part = small.tile([P, n_run], F32, tag=f"part{li}",
                                  name=f"part{li}")
                for idx, h in enumerate(range(0, pr, RUN)):
                    rl = min(RUN, pr - h)
                    off = (ljofs + h) * CH_ELS
                    wr = wrpool.tile([P, RUN * OBW], F32, tag="wr",
                                     name=f"wrA{li}_{h}")
                    dma_engs[n_dma % 2].dma_start(
                        wr[:, :rl * OBW],
                        wsh[off:off + rl * CH_ELS].rearrange(
                            "(p f) -> p f", p=P))
                    n_dma += 1
                    eng.tensor_reduce(part[:, idx:idx + 1],
                                      wr[:, :rl * OBW],
                                      mybir.AxisListType.X, OP.add,
                                      apply_absolute_value=True)
                eng.tensor_reduce(pm4[:, li:li + 1], part[:, :n_run],
                                  mybir.AxisListType.X, OP.add)
            nc.scalar.dma_start(ar_in[:], pm4[:])
            nc.gpsimd.collective_compute(
                "AllReduce", OP.add,
                ins=[ar_in.opt()], outs=[ar_out.opt()],
                replica_groups=[list(range(N_CORES))])
            pms4 = small.tile([P, 4], F32, tag="pms4", name="pms4")
            nc.scalar.dma_start(pms4[:], ar_out[:])

            # per-layer scale: partition-total via ones-matmul, mean, recip
            mwb = [None] * 4
            swb = [None] * 4
            for li in range(4):
                tot = pscale.tile([P, 1], F32, tag="pscale", name=f"ptot{li}")
                nc.tensor.matmul(tot[:], ones[:], pms4[:, li:li + 1],
                                 start=True, stop=True)
                mwl = small.tile([P, 1], F32, tag=f"mwb{li}", name=f"mwb{li}")
                nc.vector.tensor_scalar(mwl[:], tot[:], 1.0 / layers[li]["numel"],
                                        float(EPS), OP.mult, OP.max)
                swl = small.tile([P, 1], F32, tag=f"swb{li}", name=f"swb{li}")
                nc.vector.reciprocal(swl[:], mwl[:])
                mwb[li] = mwl
                swb[li] = swl

            # ---------- Stage B2: quantize + AllGather ----------
            def _quant_layer(li, on_gpsimd):
                """Re-read shard, ternary-quantize to fp8, stage + AllGather.
                on_gpsimd: ALU + DMA entirely on the gpsimd engine/queue so it
                cannot block the main-pass scalar/vector FIFOs."""
                L = layers[li]
                pr, ljofs = L["per_rank"], layer_jofs[li]
                nh = L["n_halves"]
                prh = pr // nh
                swl = swb[li]
                for half in range(nh):
                    for h in range(0, prh, RUN):
                        rl = min(RUN, prh - h)
                        hh = half * prh + h
                        off = (ljofs + hh) * CH_ELS
                        wr = wrpool.tile([P, RUN * OBW], F32, tag="wr",
                                         name=f"wrQ{li}_{hh}")
                        q8 = q8pool.tile([P, RUN * OBW], FP8, tag="q8",
                                         name=f"q8_{li}_{hh}")
                        src = wsh[off:off + rl * CH_ELS].rearrange(
                            "(p f) -> p f", p=P)
                        dst = stage[li][half][h * CH_ELS:
                                              h * CH_ELS + rl * CH_ELS
                                              ].rearrange("(p f) -> p f", p=P)
                        if on_gpsimd:
                            nc.gpsimd.dma_start(wr[:, :rl * OBW], src)
                            nc.gpsimd.tensor_scalar(wr[:, :rl * OBW],
                                                    wr[:, :rl * OBW],
                                                    swl[:], MAGIC,
                                                    OP.mult, OP.add)
                            nc.gpsimd.tensor_scalar(wr[:, :rl * OBW],
                                                    wr[:, :rl * OBW],
                                                    MAGIC, 1.0,
                                                    OP.subtract, OP.min)
                            nc.gpsimd.tensor_scalar(q8[:, :rl * OBW],
                                                    wr[:, :rl * OBW],
                                                    -1.0, None, OP.max)
                            nc.gpsimd.dma_start(dst, q8[:, :rl * OBW])
                        else:
                            dma_engs[(li + h) % 2].dma_start(
                                wr[:, :rl * OBW], src)
                            nc.scalar.activation(wr[:, :rl * OBW],
                                                 wr[:, :rl * OBW],
                                                 AF.Copy, bias=MAGIC,
                                                 scale=swl[:])
                            nc.vector.tensor_scalar(wr[:, :rl * OBW],
                                                    wr[:, :rl * OBW],
                                                    MAGIC, 1.0,
                                                    OP.subtract, OP.min)
                            nc.vector.tensor_scalar(q8[:, :rl * OBW],
                                                    wr[:, :rl * OBW],
                                                    -1.0, None, OP.max)
                            nc.scalar.dma_start(dst, q8[:, :rl * OBW])
                    nc.gpsimd.collective_compute(
                        "AllGather", OP.bypass,
                        ins=[stage[li][half].opt()],
                        outs=[image[li][half].opt()],
                        replica_groups=[list(range(N_CORES))])

            _quant_layer(0, on_gpsimd=False)
            _quant_layer(1, on_gpsimd=False)

            # per-row dequant scale for layer 0
            c_cur = []
            for t in range(T):
                c0 = small.tile([P, 1], F32, tag=f"c0_{t}", name=f"c0_{t}")
                nc.vector.scalar_tensor_tensor(c0[:], am0s[t][:], 1.0 / 127.0,
                                               mwb[0][:], OP.mult, OP.mult)
                c_cur.append(c0)

            # ---------------- Stage C: main pass ----------------
            def _main_layer(li, xqT_cur, c_cur):
                L = layers[li]
                n_ic, n_ob = L["n_ic"], L["n_ob"]
                panel_ic, n_panels = L["panel_ic"], L["n_panels"]
                dout, dreal = L["dout"], L["dreal"]
                is_last = (li == 3)

                ys = [ypool.tile([P, dreal], F32, tag="y", name=f"y{li}_{t}")
                      for t in range(T)]
                bns = [small.tile([P, n_ob * 6], F32, tag=f"bn{t}",
                                  name=f"bn{li}_{t}")
                       for t in range(T)] if not is_last else None
                if not is_last:
                    n_ic_next = layers[li + 1]["n_ic"]
                    xqT_next = [None] * T
                    c_next = [None] * T

                for s0 in range(0, T, SW):
                    ts_ = list(range(s0, min(s0 + SW, T)))
                    for ob in range(n_ob):
                        ow = L["ob_w"][ob]
                        ps = {t: psum.tile([P, OBW], F32, tag="ps",
                                           name=f"ps{li}_{ob}_{t}")
                              for t in ts_}
                        for panel in range(n_panels):
                            wp = wpool.tile([P, panel_ic, OBW], FP8, tag="wp",
                                            name=f"wp{li}_{s0}_{ob}_{panel}")
                            g0 = (ob * n_ic + panel * panel_ic)
                            pr_l = L["per_rank"]
                            prh_l = pr_l // L["n_halves"]
                            rnk, j = divmod(g0, pr_l)
                            half, jl = divmod(j, prh_l)
                            uoff = (rnk * prh_l + jl) * CH_ELS
                            nc.sync.dma_start(
                                wp[:],
                                image[li][half][uoff:uoff + panel_ic * CH_ELS]
                                .rearrange("(p c f) -> p c f", p=P, c=panel_ic))
                            for t in ts_:
                                for cc in range(panel_ic):
                                    c = panel * panel_ic + cc
                                    nc.tensor.matmul(
                                        ps[t][:, :ow], xqT_cur[t][:, c, :],
                                        wp[:, cc, :ow],
                                        start=(c == 0), stop=(c == n_ic - 1))
                        for t in ts_:
                            if not is_last:
                                dst = ys[t][:, ob * OBW:ob * OBW + ow]
                                nc.scalar.activation(dst, ps[t][:, :ow],
                                                     AF.Copy, bias=0.0,
                                                     scale=c_cur[t][:])
                                nc.vector.bn_stats(bns[t][:, ob * 6:(ob + 1) * 6],
                                                   dst)
                            else:
                                # final layer: sigmoid evict + per-ob output
                                sgo = outpool.tile([P, OBW], F32, tag="outr",
                                                   name=f"sg3_{t}_{ob}")
                                nc.scalar.activation(sgo[:, :ow], ps[t][:, :ow],
                                                     AF.Sigmoid, bias=0.0,
                                                     scale=c_cur[t][:])
                                c_lo = ob * OBW
                                # mz part: cols [0, OBINS)
                                if c_lo < OBINS:
                                    w_mz = min(ow, OBINS - c_lo)
                                    mzt = outpool.tile([P, OBW], F32,
                                                       tag="outr2",
                                                       name=f"mz3_{t}_{ob}")
                                    nc.vector.tensor_scalar(
                                        mzt[:, :w_mz], sgo[:, :w_mz],
                                        float(OBINS - 1), 1.0,
                                        OP.mult, OP.add)
                                    nc.scalar.dma_start(
                                        mz_out[t * P:(t + 1) * P,
                                               c_lo:c_lo + w_mz],
                                        mzt[:, :w_mz])
                                # ii part: cols [OBINS, 2*OBINS)
                                if c_lo + ow > OBINS:
                                    s_in = max(0, OBINS - c_lo)
                                    w_ii = ow - s_in
                                    o_lo = c_lo + s_in - OBINS
                                    iit = outpool.tile([P, OBW], F32,
                                                       tag="outr2",
                                                       name=f"ii3_{t}_{ob}")
                                    nc.vector.tensor_scalar(
                                        iit[:, :w_ii], sgo[:, s_in:s_in + w_ii],
                                        100.0, None, OP.mult)
                                    nc.scalar.dma_start(
                                        ii_out[t * P:(t + 1) * P,
                                               o_lo:o_lo + w_ii],
                                        iit[:, :w_ii])

                    if is_last:
                        continue

                    # ---- tail for this sweep: LN+SiLU (fused) + quant + T
                    for t in ts_:
                        mv = small.tile([P, 2], F32, tag="mv",
                                        name=f"mv{li}_{t}")
                        nc.vector.bn_aggr(mv[:], bns[t][:])
                        v = small.tile([P, 1], F32, tag="vvar",
                                       name=f"v{li}_{t}")
                        nc.vector.tensor_scalar(v[:], mv[:, 1:2], float(EPS),
                                                None, OP.add)
                        istd = _rsqrt_newton(nc, small, v[:])
                        nmi = small.tile([P, 1], F32, tag="nmi",
                                         name=f"nmi{li}_{t}")
                        nc.vector.scalar_tensor_tensor(nmi[:], mv[:, 0:1], -1.0,
                                                       istd[:], OP.mult, OP.mult)
                        amsl = small.tile([P, 8], F32, tag="amsl",
                                          name=f"amsl{li}_{t}")
                        n_chk = dout // OBW
                        for ch in range(n_chk):
                            sl = ys[t][:, ch * OBW:(ch + 1) * OBW]
                            if SILU_AF:
                                # h = silu((y - mu) * istd), one ACT op
                                nc.scalar.activation(sl, sl, AF.Silu,
                                                     bias=nmi[:],
                                                     scale=istd[:])
                            else:
                                nc.scalar.activation(sl, sl, AF.Identity,
                                                     bias=nmi[:],
                                                     scale=istd[:])
                                sg = sgpool.tile([P, OBW], F32, tag="sg",
                                                 name=f"sg{li}_{t}_{ch}")
                                nc.scalar.activation(sg[:], sl, AF.Tanh,
                                                     bias=0.0, scale=0.5)
                                nc.vector.tensor_scalar(sg[:], sg[:], 0.5, 0.5,
                                                        OP.mult, OP.add)
                                nc.vector.tensor_tensor(sl, sl, sg[:], OP.mult)
                            nc.vector.tensor_reduce(amsl[:, ch:ch + 1], sl,
                                                    mybir.AxisListType.X,
                                                    OP.max,
                                                    apply_absolute_value=True)
                        am = small.tile([P, 1], F32, tag="amn",
                                        name=f"am{li}_{t}")
                        nc.vector.tensor_reduce(am[:], amsl[:, :n_chk],
                                                mybir.AxisListType.X, OP.max)
                        nc.vector.tensor_scalar(am[:], am[:], float(EPS), None,
                                                OP.max)
                        sc = small.tile([P, 1], F32, tag="scn",
                                        name=f"sc{li}_{t}")
                        nc.vector.tensor_scalar(sc[:], am[:], 1.0 / 127.0,
                                                None, OP.mult)
                        nc.vector.reciprocal(sc[:], sc[:])
                        cn = small.tile([P, 1], F32, tag=f"c{li + 1}_{t}",
                                        name=f"c{li + 1}_{t}")
                        nc.vector.scalar_tensor_tensor(cn[:], am[:],
                                                       1.0 / 127.0,
                                                       mwb[li + 1][:],
                                                       OP.mult, OP.mult)
                        c_next[t] = cn
                        xqn = xqnpool.tile([P, dout], BF16, tag="xqn",
                                           name=f"xqn{li}_{t}")
                        for ch in range(n_chk):
                            uu = upool.tile([P, OBW], F32, tag="u",
                                            name=f"ur{li}_{t}_{ch}")
                            nc.scalar.activation(uu[:],
                                                 ys[t][:, ch * OBW:(ch + 1) * OBW],
                                                 AF.Copy, bias=MAGIC,
                                                 scale=sc[:])
                            nc.vector.tensor_scalar(xqn[:, ch * OBW:(ch + 1) * OBW],
                                                    uu[:], MAGIC, None,
                                                    OP.subtract)
                        xT = xqTpool.tile([P, n_ic_next, P], BF16, tag="xqT",
                                          name=f"xT{li}_{t}")
                        # alternate transposes between the scalar and sync
                        # queues: each one BLOCKS its queue for the full XBAR
                        # transfer (~4.6us for 1MB)
                        teng = nc.scalar if t % 2 == 0 else nc.sync
                        teng.dma_start_transpose(xT[:], xqn[:])
                        xqT_next[t] = xT
                if is_last:
                    return None, None
                return xqT_next, c_next

            xqT_cur, c_cur = _main_layer(0, xqT_cur, c_cur)
            _quant_layer(2, on_gpsimd=True)
            xqT_cur, c_cur = _main_layer(1, xqT_cur, c_cur)
            _quant_layer(3, on_gpsimd=True)
            xqT_cur, c_cur = _main_layer(2, xqT_cur, c_cur)
            _main_layer(3, xqT_cur, c_cur)

    nc.compile()
    return nc


def prepare_inputs(cfg, x, W0, W1, W2, W3):
    """Host-side sharding: per-core input maps. Weight chunks are shipped
    unit-major: unit u = (layer, ob, panel) is a [128, panel_ic*512] block,
    rows = partitions, contiguous per row; chunk cc of the unit holds
    W_l[ob*512+o, (panel*panel_ic+cc)*128+p] at [p, cc*512+o] (i.e. W^T)."""
    layers, total_ch, per_rank, T = _plan(cfg)
    b_core = T * P
    Ws = [np.asarray(W0), np.asarray(W1), np.asarray(W2), np.asarray(W3)]
    WTs = []
    for L, W in zip(layers, Ws):
        WT = np.zeros((L["din"], L["dout"]), dtype=np.float32)
        WT[:, :L["dreal"]] = W.T
        WTs.append(WT)

    shards = [np.empty(per_rank * CH_ELS, dtype=np.float32)
              for _ in range(N_CORES)]
    for L in layers:
        li, pr = L["li"], L["per_rank"]
        n_ic, panel_ic = L["n_ic"], L["panel_ic"]
        WT = WTs[li]
        for r in range(N_CORES):
            g0 = r * pr
            dst = shards[r]
            for j in range(0, pr, panel_ic):
                g = g0 + j
                ob, ic0 = divmod(g, n_ic)
                assert ic0 % panel_ic == 0
                # unit block [p, cc, o]
                blk = WT[ic0 * P:(ic0 + panel_ic) * P,
                         ob * OBW:(ob + 1) * OBW]          # [panel_ic*128, 512]
                blk = blk.reshape(panel_ic, P, OBW).transpose(1, 0, 2)
                off = (L["ch_base"] // N_CORES + j) * CH_ELS
                dst[off:off + panel_ic * CH_ELS] = blk.reshape(-1)
    x = np.asarray(x, dtype=np.float32)
    in_maps = []
    for r in range(N_CORES):
        in_maps.append(dict(
            xs=np.ascontiguousarray(x[r * b_core:(r + 1) * b_core]),
            wsh=shards[r],
        ))
    return in_maps


_NC_CACHE = {}


def _get_nc(cfg_key):
    if cfg_key not in _NC_CACHE:
        _NC_CACHE[cfg_key] = build(dict(cfg_key))
    return _NC_CACHE[cfg_key]


def run(cfg, x, W0, W1, W2, W3, trace=False):
    layers, total_ch, per_rank, T = _plan(cfg)
    b_core = T * P
    nc = _get_nc(tuple(sorted(cfg.items())))
    in_maps = prepare_inputs(cfg, x, W0, W1, W2, W3)
    res = run_bass_kernel_spmd(nc, in_maps, core_ids=list(range(N_CORES)),
                               trace=trace)
    mz = np.concatenate([res.results[r]["mz"] for r in range(N_CORES)], axis=0)
    ii = np.concatenate([res.results[r]["ii"] for r in range(N_CORES)], axis=0)
    return (mz, ii), res


def kernel(x, W0, W1, W2, W3, g0, b0, g1, b1, g2, b2):
    """Full-input entry point. g/b are identity (ones/zeros) in this problem's
    setup; LayerNorm affine is a no-op and is validated here."""
    for g in (g0, g1, g2):
        assert np.allclose(np.asarray(g), 1.0), "non-identity LN gain unsupported"
    for b in (b0, b1, b2):
        assert np.allclose(np.asarray(b), 0.0), "non-zero LN bias unsupported"
    (mz, ii), _ = run(FULL_CFG, x, W0, W1, W2, W3, trace=False)
    return (mz, ii)


# revision 10
# speedup vs baseline: 1.4069x; 1.4069x over previous
"""BitNet decoder MLP on 8 Trainium2 NeuronCores (Bass/Tile).

Strategy: data-parallel over batch (512 rows/core). Weights are ternary-
quantized cooperatively: each core streams its 1/8 of the fp32 weight
chunks through a |W| pass (DVE for L0/L1, gpsimd for L2/L3, reads split
across two DMA queues so the AllReduce triggers at the barrier floor),
one [128,4] AllReduce covers all four layers' global |W| sums, the
cross-partition total comes from a ones-matmul on the idle tensor
engine. L0/L1 are then re-read + quantized to fp8 ternary on scalar/DVE
and AllGathered immediately; L2/L3 quant runs entirely on the gpsimd
engine (ALU + its own DMA queue) and is EMITTED BETWEEN main-pass layer
emissions so its work never head-of-line-blocks the scalar/vector FIFOs
that the main pass needs (this blocking cost the previous revision
~200us: main-pass evictions sat behind DMA-gated quant ACTs).

Matmuls are bf16 int8-valued activations (stationary) x fp8 ternary
weights (moving), fp32 PSUM accumulation - arithmetic exact. fp8
activations (DoubleRow 2x) were measured numerically infeasible: the
reference pipeline amplifies any per-layer perturbation via act-quant
boundary flips (even an exact f32 reimplementation sits at ~1.1e-2 of
the 2e-2 budget; fp8 acts land at 2-5e-2).

Main pass is pipelined at half-batch granularity: each layer runs two
sweeps of 2 batch tiles; while sweep B's matmuls run, sweep A's tail
executes on scalar/vector engines. The tail is fused: one scalar ACT
computes silu((y-mu)*istd) directly (Silu af with per-partition
bias/scale), then absmax -> scale -> one ACT + one DVE op requantize to
bf16, and a DMA-transpose produces the next layer's stationary tiles.
Per-layer AllGathers fire as soon as each half is quantized; the CC
stream order is L0, L1h0, L1h1, L2h0, L2h1, L3 so each layer's image
lands before its matmuls need it.

Weight images are unit-major ("unit" = [128, panel_ic*512] block with
contiguous per-partition rows) so big DMAs move >=8KB-contiguous lines.
"""

from collections import deque

import numpy as np

import concourse.bass as bass
import concourse.mybir as mybir
import concourse.tile as tile
from concourse import bacc, bass_isa
from concourse.bass_utils import run_bass_kernel_spmd

F32 = mybir.dt.float32
BF16 = mybir.dt.bfloat16
FP8 = mybir.dt.float8e4
AF = mybir.ActivationFunctionType
OP = mybir.AluOpType

N_CORES = 8
P = 128
OBW = 512            # output block width (one PSUM bank of fp32)
CH_ELS = P * OBW     # elements per weight chunk
MAGIC = 12582912.0   # 1.5 * 2**23: fp32 round-to-nearest-even trick
EPS = 1e-5
RUN = 4              # chunks per weight-pass slice ([128, 2048] f32)
SILU_AF = True       # fused Silu activation table (False: tanh compose)

FULL_CFG = dict(B=4096, D0=1024, H=4096, OBINS=1000)


def _plan(cfg):
    """Static per-layer plan."""
    B, D0, H, OBINS = cfg["B"], cfg["D0"], cfg["H"], cfg["OBINS"]
    o3_real = 2 * OBINS
    o3_pad = ((o3_real + OBW - 1) // OBW) * OBW
    dims = [
        dict(din=D0, dout=H, dreal=H),
        dict(din=H, dout=H, dreal=H),
        dict(din=H, dout=H, dreal=H),
        dict(din=H, dout=o3_pad, dreal=o3_real),
    ]
    numels = [H * D0, H * H, H * H, o3_real * H]  # real numels for mean|W|
    layers = []
    ch_base = 0
    for li, d in enumerate(dims):
        n_ic = d["din"] // P
        n_ob = d["dout"] // OBW
        n_ch = n_ob * n_ic
        assert n_ch % N_CORES == 0, (li, n_ch)
        panel_ic = min(16, n_ic, max(1, n_ch // N_CORES))
        assert n_ic % panel_ic == 0 and (n_ch // N_CORES) % panel_ic == 0
        n_panels = n_ic // panel_ic
        layers.append(dict(
            li=li, din=d["din"], dout=d["dout"], dreal=d["dreal"],
            n_ic=n_ic, n_ob=n_ob, n_ch=n_ch, per_rank=n_ch // N_CORES,
            panel_ic=panel_ic, n_panels=n_panels,
            numel=numels[li], ch_base=ch_base,
            n_halves=2 if (n_ch // N_CORES) % 2 == 0 and (n_ch // N_CORES) // 2 % panel_ic == 0 else 1,
            ob_w=[min(OBW, d["dreal"] - ob * OBW) for ob in range(n_ob)],
        ))
        ch_base += n_ch
    total_ch = ch_base
    per_rank = total_ch // N_CORES
    b_core = B // N_CORES
    assert b_core % P == 0
    return layers, total_ch, per_rank, b_core // P


def _rsqrt_newton(nc, pool, v, n_iter=3):
    """istd = 1/sqrt(v) for v [128,1] fp32 (v > 0), pure-DVE Newton iteration."""
    seed = pool.tile([P, 1], F32, tag="rs_seed", name="rs_seed")
    seed_i32 = seed[:].bitcast(mybir.dt.int32)
    v_i32 = v.bitcast(mybir.dt.int32)
    nc.vector.tensor_scalar(seed_i32[:], v_i32[:], -0.5,
                            float(0x5F370000), OP.mult, OP.add)
    y = seed
    t1 = pool.tile([P, 1], F32, tag="rs_t1", name="rs_t1")
    t2 = pool.tile([P, 1], F32, tag="rs_t2", name="rs_t2")
    for _ in range(n_iter):
        nc.vector.tensor_tensor(t1[:], y[:], y[:], OP.mult)
        nc.vector.tensor_tensor(t2[:], t1[:], v, OP.mult)
        nc.vector.tensor_scalar(t1[:], t2[:], -0.5, 1.5, OP.mult, OP.add)
        nc.vector.tensor_tensor(y[:], y[:], t1[:], OP.mult)
    return y


def build(cfg):
    layers, total_ch, per_rank, T = _plan(cfg)
    nc = bacc.Bacc("TRN2", target_bir_lowering=False, debug=False,
                   num_devices=N_CORES)

    D0, OBINS = cfg["D0"], cfg["OBINS"]
    b_core = T * P
    SW = min(2, T)  # batch tiles per PE sweep

    xs = nc.dram_tensor("xs", [b_core, D0], F32, kind="ExternalInput")
    # rank's weight chunks, unit-major flat fp32 (see prepare_inputs)
    wsh = nc.dram_tensor("wsh", [per_rank * CH_ELS], F32, kind="ExternalInput")
    mz_out = nc.dram_tensor("mz", [b_core, OBINS], F32, kind="ExternalOutput")
    ii_out = nc.dram_tensor("ii", [b_core, OBINS], F32, kind="ExternalOutput")

    with tile.TileContext(nc) as tc:
        with (
            tc.tile_pool(name="ybig", bufs=4) as ypool,        # 16KB/part f32
            tc.tile_pool(name="wr", bufs=3) as wrpool,         # [128,2048] f32 runs
            tc.tile_pool(name="xqT", bufs=4) as xqTpool,       # [128,32,128] bf16
            tc.tile_pool(name="xqT0", bufs=4) as xqT0pool,     # [128,n_ic0,128] bf16
            tc.tile_pool(name="wp", bufs=3) as wpool,          # [128,16,512] fp8
            tc.tile_pool(name="xqn", bufs=2) as xqnpool,       # 8KB/partition bf16
            tc.tile_pool(name="u", bufs=2) as upool,           # [128,512] f32
            tc.tile_pool(name="sg", bufs=2) as sgpool,         # [128,512] f32
            tc.tile_pool(name="q8", bufs=2) as q8pool,         # [128,2048] fp8
            tc.tile_pool(name="outr", bufs=2) as outpool,      # [128,512] f32
            tc.tile_pool(name="small", bufs=1) as small,
            tc.tile_pool(name="psum", bufs=7, space="PSUM") as psum,
            tc.tile_pool(name="psc", bufs=1, space="PSUM") as pscale,
            tc.tile_pool(name="dram", bufs=1, space="DRAM") as dram,
        ):
            # ---------------- DRAM scratch (flat, unit-major) ----------------
            stage = []
            image = []
            for L in layers:
                nh = L["n_halves"]
                hs = L["per_rank"] // nh * CH_ELS
                hi = L["n_ch"] // nh * CH_ELS
                stage.append([dram.tile([hs], FP8, tag=f"stage{L['li']}_{h}",
                                        name=f"stage{L['li']}_{h}")
                              for h in range(nh)])
                image.append([dram.tile([hi], FP8, tag=f"image{L['li']}_{h}",
                                        name=f"image{L['li']}_{h}",
                                        addr_space="Shared")
                              for h in range(nh)])
            ar_in = dram.tile([P, 4], F32, tag="ar_in", name="ar_in")
            ar_out = dram.tile([P, 4], F32, tag="ar_out", name="ar_out",
                               addr_space="Shared")

            layer_jofs = {}
            jofs = 0
            for L in layers:
                layer_jofs[L["li"]] = jofs
                jofs += L["per_rank"]

            # ---------------- Stage A: input activation quant ----------------
            # x loads ride the scalar DMA queue so the sync queue belongs to
            # the weight-abs reads from t=0.
            n_ic0 = layers[0]["n_ic"]
            xqT_cur = []
            am0s = []
            for t in range(T):
                xt = ypool.tile([P, D0], F32, tag="y", name=f"xt{t}")
                nc.scalar.dma_start(xt[:], xs[t * P:(t + 1) * P, :])
                am = small.tile([P, 1], F32, tag=f"am0_{t}", name=f"am0_{t}")
                nc.vector.tensor_reduce(am[:], xt[:], mybir.AxisListType.X,
                                        OP.max, apply_absolute_value=True)
                nc.vector.tensor_scalar(am[:], am[:], float(EPS), None, OP.max)
                sc = small.tile([P, 1], F32, tag=f"s0_{t}", name=f"s0_{t}")
                nc.vector.tensor_scalar(sc[:], am[:], 1.0 / 127.0, None, OP.mult)
                nc.vector.reciprocal(sc[:], sc[:])
                xq0 = xqnpool.tile([P, D0], BF16, tag="xqn", name=f"xq0_{t}")
                for ch in range(D0 // OBW):
                    uu = upool.tile([P, OBW], F32, tag="u", name=f"u0_{t}_{ch}")
                    nc.scalar.activation(uu[:], xt[:, ch * OBW:(ch + 1) * OBW],
                                         AF.Copy, bias=MAGIC, scale=sc[:])
                    nc.vector.tensor_scalar(xq0[:, ch * OBW:(ch + 1) * OBW],
                                            uu[:], MAGIC, None, OP.subtract)
                xqT0 = xqT0pool.tile([P, n_ic0, P], BF16, tag="xqT0",
                                     name=f"xqT0_{t}")
                nc.scalar.dma_start_transpose(xqT0[:], xq0[:])
                xqT_cur.append(xqT0)
                am0s.append(am)

            # ---------- Stage B1: |W| pass for all layers ----------
            # Reads alternate between the sync and scalar DMA queues (a single
            # queue tops out ~220GB/s; two get the 23MB shard read done by
            # ~worst(core-start skew)). L0/L1 reduce on DVE, L2/L3 on gpsimd.
            ones = small.tile([P, P], F32, tag="ones", name="ones")
            nc.vector.memset(ones[:], 1.0)

            pm4 = small.tile([P, 4], F32, tag="pm4", name="pm4")
            dma_engs = [nc.sync, nc.scalar, nc.gpsimd]
            n_dma = 0
            for li in range(4):
                L = layers[li]
                pr, ljofs = L["per_rank"], layer_jofs[li]
                eng = nc.vector  # gpsimd tensor_reduce lacks free-axis mode
                n_run = (pr + RUN - 1) // RUN
                part = small.tile([P, n_run], F32, tag=f"part{li}",
                                  name=f"part{li}")
                for idx, h in enumerate(range(0, pr, RUN)):
                    rl = min(RUN, pr - h)
                    off = (ljofs + h) * CH_ELS
                    wr = wrpool.tile([P, RUN * OBW], F32, tag="wr",
                                     name=f"wrA{li}_{h}")
                    dma_engs[n_dma % 3].dma_start(
                        wr[:, :rl * OBW],
                        wsh[off:off + rl * CH_ELS].rearrange(
                            "(p f) -> p f", p=P))
                    n_dma += 1
                    eng.tensor_reduce(part[:, idx:idx + 1],
                                      wr[:, :rl * OBW],
                                      mybir.AxisListType.X, OP.add,
                                      apply_absolute_value=True)
                eng.tensor_reduce(pm4[:, li:li + 1], part[:, :n_run],
                                  mybir.AxisListType.X, OP.add)
            nc.scalar.dma_start(ar_in[:], pm4[:])
            nc.gpsimd.collective_compute(
                "AllReduce", OP.add,
                ins=[ar_in.opt()], outs=[ar_out.opt()],
                replica_groups=[list(range(N_CORES))])
            pms4 = small.tile([P, 4], F32, tag="pms4", name="pms4")
            nc.scalar.dma_start(pms4[:], ar_out[:])

            # per-layer scale: partition-total via ones-matmul, mean, recip
            mwb = [None] * 4
            swb = [None] * 4
            for li in range(4):
                tot = pscale.tile([P, 1], F32, tag="pscale", name=f"ptot{li}")
                nc.tensor.matmul(tot[:], ones[:], pms4[:, li:li + 1],
                                 start=True, stop=True)
                mwl = small.tile([P, 1], F32, tag=f"mwb{li}", name=f"mwb{li}")
                nc.vector.tensor_scalar(mwl[:], tot[:], 1.0 / layers[li]["numel"],
                                        float(EPS), OP.mult, OP.max)
                swl = small.tile([P, 1], F32, tag=f"swb{li}", name=f"swb{li}")
                nc.vector.reciprocal(swl[:], mwl[:])
                mwb[li] = mwl
                swb[li] = swl

            # ---------- Stage B2: quantize + AllGather ----------
            def _quant_dma(li, half, h, late):
                """Issue the shard re-read for one run; returns the tile."""
                L = layers[li]
                prh = L["per_rank"] // L["n_halves"]
                rl = min(RUN, prh - h)
                hh = half * prh + h
                off = (layer_jofs[li] + hh) * CH_ELS
                wr = wrpool.tile([P, RUN * OBW], F32, tag="wr",
                                 name=f"wrQ{li}_{hh}")
                src = wsh[off:off + rl * CH_ELS].rearrange("(p f) -> p f", p=P)
                rd_eng = nc.gpsimd if late else dma_engs[(li + h) % 2]
                rd_eng.dma_start(wr[:, :rl * OBW], src)
                return wr

            def _quant_alu(li, half, h, wr, late):
                """Ternary-quantize a fetched run on scalar+DVE, stage it."""
                L = layers[li]
                prh = L["per_rank"] // L["n_halves"]
                rl = min(RUN, prh - h)
                q8 = q8pool.tile([P, RUN * OBW], FP8, tag="q8",
                                 name=f"q8_{li}_{half}_{h}")
                dst = stage[li][half][h * CH_ELS:h * CH_ELS + rl * CH_ELS
                                      ].rearrange("(p f) -> p f", p=P)
                nc.scalar.activation(wr[:, :rl * OBW], wr[:, :rl * OBW],
                                     AF.Copy, bias=MAGIC, scale=swb[li][:])
                nc.vector.tensor_scalar(wr[:, :rl * OBW], wr[:, :rl * OBW],
                                        MAGIC, 1.0, OP.subtract, OP.min)
                nc.vector.tensor_scalar(q8[:, :rl * OBW], wr[:, :rl * OBW],
                                        -1.0, None, OP.max)
                wr_eng = nc.gpsimd if late else nc.scalar
                wr_eng.dma_start(dst, q8[:, :rl * OBW])

            def _ag_trigger(li, half):
                nc.gpsimd.collective_compute(
                    "AllGather", OP.bypass,
                    ins=[stage[li][half].opt()],
                    outs=[image[li][half].opt()],
                    replica_groups=[list(range(N_CORES))])

            def _quant_layer_tasks(li, late, pre=2):
                """Task closures: each ALU task also issues the DMA `pre`
                runs ahead so the scalar FIFO never stalls on a fetch."""
                L = layers[li]
                prh = L["per_rank"] // L["n_halves"]
                runs = [(half, h) for half in range(L["n_halves"])
                        for h in range(0, prh, RUN)]
                pending = {}

                def _fetch(i):
                    if i < len(runs):
                        half, h = runs[i]
                        pending[i] = _quant_dma(li, half, h, late)

                tasks = [lambda i=i: _fetch(i) for i in range(min(pre, len(runs)))]
                for i, (half, h) in enumerate(runs):
                    def alu_task(i=i, half=half, h=h):
                        _quant_alu(li, half, h, pending.pop(i), late)
                        _fetch(i + pre)
                    tasks.append(alu_task)
                    if h + RUN >= prh:  # last run of this half
                        tasks.append(
                            lambda li=li, half=half: _ag_trigger(li, half))
                return tasks

            for t_ in _quant_layer_tasks(0, late=False):
                t_()
            for t_ in _quant_layer_tasks(1, late=False):
                t_()

            # per-row dequant scale for layer 0
            c_cur = []
            for t in range(T):
                c0 = small.tile([P, 1], F32, tag=f"c0_{t}", name=f"c0_{t}")
                nc.vector.scalar_tensor_tensor(c0[:], am0s[t][:], 1.0 / 127.0,
                                               mwb[0][:], OP.mult, OP.mult)
                c_cur.append(c0)

            # ---------------- Stage C: main pass ----------------
            def _main_layer(li, xqT_cur, c_cur, bg=None):
                """bg: deque of background-task closures (late weight-quant
                runs + AG triggers); one is emitted after each (sweep, ob)
                eviction block so its scalar/DVE work lands in queue slots
                where the main pass has slack and its DMA prefetches an ob
                ahead."""
                L = layers[li]
                n_ic, n_ob = L["n_ic"], L["n_ob"]
                panel_ic, n_panels = L["panel_ic"], L["n_panels"]
                dout, dreal = L["dout"], L["dreal"]
                is_last = (li == 3)

                ys = [ypool.tile([P, dreal], F32, tag="y", name=f"y{li}_{t}")
                      for t in range(T)] if not is_last else None
                bns = [small.tile([P, n_ob * 6], F32, tag=f"bn{t}",
                                  name=f"bn{li}_{t}")
                       for t in range(T)] if not is_last else None
                if not is_last:
                    n_ic_next = layers[li + 1]["n_ic"]
                    xqT_next = [None] * T
                    c_next = [None] * T

                for s0 in range(0, T, SW):
                    ts_ = list(range(s0, min(s0 + SW, T)))
                    for ob in range(n_ob):
                        ow = L["ob_w"][ob]
                        ps = {t: psum.tile([P, OBW], F32, tag="ps",
                                           name=f"ps{li}_{ob}_{t}")
                              for t in ts_}
                        for panel in range(n_panels):
                            wp = wpool.tile([P, panel_ic, OBW], FP8, tag="wp",
                                            name=f"wp{li}_{s0}_{ob}_{panel}")
                            g0 = (ob * n_ic + panel * panel_ic)
                            pr_l = L["per_rank"]
                            prh_l = pr_l // L["n_halves"]
                            rnk, j = divmod(g0, pr_l)
                            half, jl = divmod(j, prh_l)
                            uoff = (rnk * prh_l + jl) * CH_ELS
                            nc.sync.dma_start(
                                wp[:],
                                image[li][half][uoff:uoff + panel_ic * CH_ELS]
                                .rearrange("(p c f) -> p c f", p=P, c=panel_ic))
                            for t in ts_:
                                for cc in range(panel_ic):
                                    c = panel * panel_ic + cc
                                    nc.tensor.matmul(
                                        ps[t][:, :ow], xqT_cur[t][:, c, :],
                                        wp[:, cc, :ow],
                                        start=(c == 0), stop=(c == n_ic - 1))
                        for t in ts_:
                            if not is_last:
                                dst = ys[t][:, ob * OBW:ob * OBW + ow]
                                nc.scalar.activation(dst, ps[t][:, :ow],
                                                     AF.Copy, bias=0.0,
                                                     scale=c_cur[t][:])
                                nc.vector.bn_stats(bns[t][:, ob * 6:(ob + 1) * 6],
                                                   dst)
                            else:
                                # final layer: sigmoid evict + per-ob output
                                sgo = outpool.tile([P, OBW], F32, tag="outr",
                                                   name=f"sg3_{t}_{ob}")
                                nc.scalar.activation(sgo[:, :ow], ps[t][:, :ow],
                                                     AF.Sigmoid, bias=0.0,
                                                     scale=c_cur[t][:])
                                c_lo = ob * OBW
                                # mz part: cols [0, OBINS)
                                if c_lo < OBINS:
                                    w_mz = min(ow, OBINS - c_lo)
                                    mzt = outpool.tile([P, OBW], F32,
                                                       tag="outr2",
                                                       name=f"mz3_{t}_{ob}")
                                    nc.vector.tensor_scalar(
                                        mzt[:, :w_mz], sgo[:, :w_mz],
                                        float(OBINS - 1), 1.0,
                                        OP.mult, OP.add)
                                    nc.scalar.dma_start(
                                        mz_out[t * P:(t + 1) * P,
                                               c_lo:c_lo + w_mz],
                                        mzt[:, :w_mz])
                                # ii part: cols [OBINS, 2*OBINS)
                                if c_lo + ow > OBINS:
                                    s_in = max(0, OBINS - c_lo)
                                    w_ii = ow - s_in
                                    o_lo = c_lo + s_in - OBINS
                                    iit = outpool.tile([P, OBW], F32,
                                                       tag="outr2",
                                                       name=f"ii3_{t}_{ob}")
                                    nc.vector.tensor_scalar(
                                        iit[:, :w_ii], sgo[:, s_in:s_in + w_ii],
                                        100.0, None, OP.mult)
                                    nc.scalar.dma_start(
                                        ii_out[t * P:(t + 1) * P,
                                               o_lo:o_lo + w_ii],
                                        iit[:, :w_ii])
                        if bg:
                            bg.popleft()()

                    if is_last:
                        continue

                    # ---- tail for this sweep: LN+SiLU (fused) + quant + T
                    for t in ts_:
                        mv = small.tile([P, 2], F32, tag="mv",
                                        name=f"mv{li}_{t}")
                        nc.vector.bn_aggr(mv[:], bns[t][:])
                        v = small.tile([P, 1], F32, tag="vvar",
                                       name=f"v{li}_{t}")
                        nc.vector.tensor_scalar(v[:], mv[:, 1:2], float(EPS),
                                                None, OP.add)
                        istd = _rsqrt_newton(nc, small, v[:])
                        nmi = small.tile([P, 1], F32, tag="nmi",
                                         name=f"nmi{li}_{t}")
                        nc.vector.scalar_tensor_tensor(nmi[:], mv[:, 0:1], -1.0,
                                                       istd[:], OP.mult, OP.mult)
                        amsl = small.tile([P, 8], F32, tag="amsl",
                                          name=f"amsl{li}_{t}")
                        n_chk = dout // OBW
                        for ch in range(n_chk):
                            sl = ys[t][:, ch * OBW:(ch + 1) * OBW]
                            if SILU_AF:
                                # h = silu((y - mu) * istd), one ACT op
                                nc.scalar.activation(sl, sl, AF.Silu,
                                                     bias=nmi[:],
                                                     scale=istd[:])
                            else:
                                nc.scalar.activation(sl, sl, AF.Identity,
                                                     bias=nmi[:],
                                                     scale=istd[:])
                                sg = sgpool.tile([P, OBW], F32, tag="sg",
                                                 name=f"sg{li}_{t}_{ch}")
                                nc.scalar.activation(sg[:], sl, AF.Tanh,
                                                     bias=0.0, scale=0.5)
                                nc.vector.tensor_scalar(sg[:], sg[:], 0.5, 0.5,
                                                        OP.mult, OP.add)
                                nc.vector.tensor_tensor(sl, sl, sg[:], OP.mult)
                            nc.vector.tensor_reduce(amsl[:, ch:ch + 1], sl,
                                                    mybir.AxisListType.X,
                                                    OP.max,
                                                    apply_absolute_value=True)
                        am = small.tile([P, 1], F32, tag="amn",
                                        name=f"am{li}_{t}")
                        nc.vector.tensor_reduce(am[:], amsl[:, :n_chk],
                                                mybir.AxisListType.X, OP.max)
                        nc.vector.tensor_scalar(am[:], am[:], float(EPS), None,
                                                OP.max)
                        sc = small.tile([P, 1], F32, tag="scn",
                                        name=f"sc{li}_{t}")
                        nc.vector.tensor_scalar(sc[:], am[:], 1.0 / 127.0,
                                                None, OP.mult)
                        nc.vector.reciprocal(sc[:], sc[:])
                        cn = small.tile([P, 1], F32, tag=f"c{li + 1}_{t}",
                                        name=f"c{li + 1}_{t}")
                        nc.vector.scalar_tensor_tensor(cn[:], am[:],
                                                       1.0 / 127.0,
                                                       mwb[li + 1][:],
                                                       OP.mult, OP.mult)
                        c_next[t] = cn
                        xqn = xqnpool.tile([P, dout], BF16, tag="xqn",
                                           name=f"xqn{li}_{t}")
                        for ch in range(n_chk):
                            uu = upool.tile([P, OBW], F32, tag="u",
                                            name=f"ur{li}_{t}_{ch}")
                            nc.scalar.activation(uu[:],
                                                 ys[t][:, ch * OBW:(ch + 1) * OBW],
                                                 AF.Copy, bias=MAGIC,
                                                 scale=sc[:])
                            nc.vector.tensor_scalar(xqn[:, ch * OBW:(ch + 1) * OBW],
                                                    uu[:], MAGIC, None,
                                                    OP.subtract)
                        xT = xqTpool.tile([P, n_ic_next, P], BF16, tag="xqT",
                                          name=f"xT{li}_{t}")
                        # alternate transposes between the scalar and sync
                        # queues: each one BLOCKS its queue for the full XBAR
                        # transfer (~4.6us for 1MB)
                        teng = nc.scalar if t % 2 == 0 else nc.sync
                        teng.dma_start_transpose(xT[:], xqn[:])
                        xqT_next[t] = xT
                if is_last:
                    return None, None
                return xqT_next, c_next

            # late weight-quant work drains one task per (sweep, ob) slot of
            # the earlier main layers; leftovers flush at layer boundaries
            bg = deque(_quant_layer_tasks(2, late=True)
                       + _quant_layer_tasks(3, late=True))
            xqT_cur, c_cur = _main_layer(0, xqT_cur, c_cur, bg)
            xqT_cur, c_cur = _main_layer(1, xqT_cur, c_cur, bg)
            while bg:
                bg.popleft()()
            xqT_cur, c_cur = _main_layer(2, xqT_cur, c_cur)
            _main_layer(3, xqT_cur, c_cur)

    nc.compile()
    return nc


def prepare_inputs(cfg, x, W0, W1, W2, W3):
    """Host-side sharding: per-core input maps. Weight chunks are shipped
    unit-major: unit u = (layer, ob, panel) is a [128, panel_ic*512] block,
    rows = partitions, contiguous per row; chunk cc of the unit holds
    W_l[ob*512+o, (panel*panel_ic+cc)*128+p] at [p, cc*512+o] (i.e. W^T)."""
    layers, total_ch, per_rank, T = _plan(cfg)
    b_core = T * P
    Ws = [np.asarray(W0), np.asarray(W1), np.asarray(W2), np.asarray(W3)]
    WTs = []
    for L, W in zip(layers, Ws):
        WT = np.zeros((L["din"], L["dout"]), dtype=np.float32)
        WT[:, :L["dreal"]] = W.T
        WTs.append(WT)

    shards = [np.empty(per_rank * CH_ELS, dtype=np.float32)
              for _ in range(N_CORES)]
    for L in layers:
        li, pr = L["li"], L["per_rank"]
        n_ic, panel_ic = L["n_ic"], L["panel_ic"]
        WT = WTs[li]
        for r in range(N_CORES):
            g0 = r * pr
            dst = shards[r]
            for j in range(0, pr, panel_ic):
                g = g0 + j
                ob, ic0 = divmod(g, n_ic)
                assert ic0 % panel_ic == 0
                # unit block [p, cc, o]
                blk = WT[ic0 * P:(ic0 + panel_ic) * P,
                         ob * OBW:(ob + 1) * OBW]          # [panel_ic*128, 512]
                blk = blk.reshape(panel_ic, P, OBW).transpose(1, 0, 2)
                off = (L["ch_base"] // N_CORES + j) * CH_ELS
                dst[off:off + panel_ic * CH_ELS] = blk.reshape(-1)
    x = np.asarray(x, dtype=np.float32)
    in_maps = []
    for r in range(N_CORES):
        in_maps.append(dict(
            xs=np.ascontiguousarray(x[r * b_core:(r + 1) * b_core]),
            wsh=shards[r],
        ))
    return in_maps


_NC_CACHE = {}


def _get_nc(cfg_key):
    if cfg_key not in _NC_CACHE:
        _NC_CACHE[cfg_key] = build(dict(cfg_key))
    return _NC_CACHE[cfg_key]


def run(cfg, x, W0, W1, W2, W3, trace=False):
    layers, total_ch, per_rank, T = _plan(cfg)
    b_core = T * P
    nc = _get_nc(tuple(sorted(cfg.items())))
    in_maps = prepare_inputs(cfg, x, W0, W1, W2, W3)
    res = run_bass_kernel_spmd(nc, in_maps, core_ids=list(range(N_CORES)),
                               trace=trace)
    mz = np.concatenate([res.results[r]["mz"] for r in range(N_CORES)], axis=0)
    ii = np.concatenate([res.results[r]["ii"] for r in range(N_CORES)], axis=0)
    return (mz, ii), res


def kernel(x, W0, W1, W2, W3, g0, b0, g1, b1, g2, b2):
    """Full-input entry point. g/b are identity (ones/zeros) in this problem's
    setup; LayerNorm affine is a no-op and is validated here."""
    for g in (g0, g1, g2):
        assert np.allclose(np.asarray(g), 1.0), "non-identity LN gain unsupported"
    for b in (b0, b1, b2):
        assert np.allclose(np.asarray(b), 0.0), "non-zero LN bias unsupported"
    (mz, ii), _ = run(FULL_CFG, x, W0, W1, W2, W3, trace=False)
    return (mz, ii)
